# revision 33
# baseline (speedup 1.0000x reference)
import sys
sys.path.insert(0, "/opt/trn_rl_repo")
import contextlib
import numpy as np

import concourse.bass as bass
import concourse.mybir as mybir
import concourse.tile as tile
from concourse import bacc
from concourse.masks import make_identity
from concourse.bass_utils import run_bass_kernel_spmd

F32 = mybir.dt.float32
F32R = mybir.dt.float32r
ALU = mybir.AluOpType
ACTF = mybir.ActivationFunctionType
EPS = 1e-5
MAG_EPS = 1e-12
import os
NCORES = 8
NPER = int(os.environ.get("KERNEL_NPER", "64"))  # images per core
NTOT = NCORES * NPER  # global batch

LCFG = {
    1: dict(C=16, HO=64),
    2: dict(C=32, HO=32),
    3: dict(C=64, HO=16),
    4: dict(C=64, HO=8),
}


def ap_of(t, offset, dims):
    """AP over DRAM tensor t with explicit [step,count] dims (elements)."""
    return bass.AP(tensor=t[:].tensor, offset=offset, ap=[list(d) for d in dims])


def build():
    nc = bacc.Bacc("TRN2", target_bir_lowering=False, debug=False,
                   num_devices=NCORES)

    # ---------------- parameters ----------------
    P = {}
    P["x"] = nc.declare_dram_parameter("x", [NPER, 1, 64, 64], F32, isOutput=False)
    for name, cin, cout, k in [("c1", 1, 16, 11), ("c2", 16, 32, 5),
                               ("c3", 32, 64, 3), ("c4", 64, 64, 3)]:
        P[f"{name}_wr"] = nc.declare_dram_parameter(f"{name}_wr", [cout, cin, k, k], F32, isOutput=False)
        P[f"{name}_wi"] = nc.declare_dram_parameter(f"{name}_wi", [cout, cin, k, k], F32, isOutput=False)
        P[f"{name}_br"] = nc.declare_dram_parameter(f"{name}_br", [cout], F32, isOutput=False)
        P[f"{name}_bi"] = nc.declare_dram_parameter(f"{name}_bi", [cout], F32, isOutput=False)
    for name, c in [("b1", 16), ("b2", 32), ("b3", 64), ("b4", 64)]:
        for g in ["grr", "gri", "gii", "br", "bi"]:
            P[f"{name}_{g}"] = nc.declare_dram_parameter(f"{name}_{g}", [c], F32, isOutput=False)
    P["fc1_wr"] = nc.declare_dram_parameter("fc1_wr", [256, 1024], F32, isOutput=False)
    P["fc1_wi"] = nc.declare_dram_parameter("fc1_wi", [256, 1024], F32, isOutput=False)
    P["fc1_br"] = nc.declare_dram_parameter("fc1_br", [256], F32, isOutput=False)
    P["fc1_bi"] = nc.declare_dram_parameter("fc1_bi", [256], F32, isOutput=False)
    P["fc2_w"] = nc.declare_dram_parameter("fc2_w", [1000, 256], F32, isOutput=False)
    P["fc2_b"] = nc.declare_dram_parameter("fc2_b", [1000], F32, isOutput=False)

    out_logits = nc.declare_dram_parameter("logits", [NPER, 1000], F32, isOutput=True)
    out_er = nc.declare_dram_parameter("er", [NPER, 256], F32, isOutput=True)
    out_ei = nc.declare_dram_parameter("ei", [NPER, 256], F32, isOutput=True)
    DEBUG = bool(int(os.environ.get("KERNEL_DEBUG", "0")))
    dbg = {}
    if DEBUG:
        dbg["a1"] = nc.declare_dram_parameter("dbg_a1", [2, 16, NPER, 4096], F32, isOutput=True)
        dbg["p1"] = nc.declare_dram_parameter("dbg_p1", [2, 16, NPER, 36, 36], F32, isOutput=True)
        dbg["a2"] = nc.declare_dram_parameter("dbg_a2", [2, 32, NPER, 1024], F32, isOutput=True)
        dbg["p4"] = nc.declare_dram_parameter("dbg_p4", [2, NPER, 1024], F32, isOutput=True)
        for l in (1, 2, 3, 4):
            C = LCFG[l]["C"]
            dbg[f"cc{l}"] = nc.declare_dram_parameter(f"dbg_cc{l}", [5, C], F32, isOutput=True)
            dbg[f"co{l}"] = nc.declare_dram_parameter(f"dbg_co{l}", [64 * 6], F32, isOutput=True)

    # ---------------- internal DRAM ----------------
    xpad = nc.dram_tensor("xpad", [NPER, 74, 74], F32)
    a = {1: nc.dram_tensor("a1", [2, 16, NPER, 4096], F32),
         2: nc.dram_tensor("a2", [2, 32, NPER, 1024], F32),
         3: nc.dram_tensor("a3", [2, 64, NPER, 256], F32),
         4: nc.dram_tensor("a4", [2, 64, NPER, 64], F32)}
    p = {1: nc.dram_tensor("p1", [2, 16, NPER, 36, 36], F32),
         2: nc.dram_tensor("p2", [2, 32, NPER, 18, 18], F32),
         3: nc.dram_tensor("p3", [2, 64, NPER, 10, 10], F32),
         4: nc.dram_tensor("p4", [2, NPER, 1024], F32)}
    st1_dram = nc.dram_tensor("st1_dram", [121, 64], F32)
    stm_dram = {2: nc.dram_tensor("stm2", [5, 2, 80, 96], F32),
                3: nc.dram_tensor("stm3", [3, 2, 96, 128], F32),
                4: nc.dram_tensor("stm4", [3, 2, 192, 128], F32)}
    sts_dram = {3: nc.dram_tensor("sts3", [3, 2, 96, 64], F32),
                4: nc.dram_tensor("sts4", [3, 2, 192, 64], F32)}
    fc1_wineg = nc.dram_tensor("fc1_wineg", [256, 1024], F32)
    bias_dram = {l: nc.dram_tensor(f"biasd{l}", [256], F32) for l in (1, 2, 3, 4)}
    coef_dram = {l: nc.dram_tensor(f"coefd{l}", [64 * 6], F32) for l in (1, 2, 3, 4)}
    fcb_dram = nc.dram_tensor("fcbd", [512], F32)
    cc_in = {l: nc.dram_tensor(f"ccin{l}", [5, LCFG[l]["C"]], F32) for l in (1, 2, 3, 4)}
    cc_out = {l: nc.dram_tensor(f"ccout{l}", [5, LCFG[l]["C"]], F32,
                                addr_space="Shared") for l in (1, 2, 3, 4)}
    rg = [list(range(NCORES))]

    with tile.TileContext(nc) as tc:
        ctx = contextlib.ExitStack()
        with ctx:
            singles = ctx.enter_context(tc.tile_pool(name="singles", bufs=1))
            prep = ctx.enter_context(tc.tile_pool(name="prep", bufs=1))
            mvp = ctx.enter_context(tc.tile_pool(name="mv", bufs=2))
            spillp = ctx.enter_context(tc.tile_pool(name="spill", bufs=2))
            sqp = ctx.enter_context(tc.tile_pool(name="sq", bufs=1))
            statp = ctx.enter_context(tc.tile_pool(name="stat", bufs=1))
            dp = ctx.enter_context(tc.tile_pool(name="dpool", bufs=2))
            dtmp = ctx.enter_context(tc.tile_pool(name="dtmp", bufs=1))
            dout = ctx.enter_context(tc.tile_pool(name="dout", bufs=2))

            # ============ zero padded tensors ============
            zt = singles.tile([128, 1024], F32)
            nc.vector.memset(zt[:], 0.0)

            def zero_dram(t, total):
                done = 0
                while done < total:
                    nelem = min(128 * 1024, total - done)
                    rows = max(1, nelem // 1024)
                    cols = min(1024, nelem // rows)
                    nc.sync.dma_start(out=ap_of(t, done, [[1, rows * cols]]),
                                      in_=zt[0:rows, 0:cols])
                    done += rows * cols
                    if total - done > 0 and total - done < 1024:
                        nc.sync.dma_start(out=ap_of(t, done, [[1, total - done]]),
                                          in_=zt[0:1, 0:total - done])
                        done = total

            zero_dram(xpad, NPER * 74 * 74)
            zero_dram(p[1], 2 * 16 * NPER * 36 * 36)
            zero_dram(p[2], 2 * 32 * NPER * 18 * 18)
            zero_dram(p[3], 2 * 64 * NPER * 10 * 10)

            # copy x interior into xpad
            for n0 in range(0, NPER, 16):
                xin = prep.tile([64, 16, 64], F32, tag="xin")
                nc.sync.dma_start(
                    out=xin[:],
                    in_=ap_of(P["x"], n0 * 4096, [[64, 64], [4096, 16], [1, 64]]))
                nc.sync.dma_start(
                    out=ap_of(xpad, n0 * 5476 + 5 * 74 + 5,
                              [[74, 64], [5476, 16], [1, 64]]),
                    in_=xin[:])

            # ============ weight prep: build staged stationaries ============
            zero_dram(st1_dram, 121 * 64)
            for l in (2, 3, 4):
                cout = P[f"c{l}_wr"][:].shape[0]
                cin = P[f"c{l}_wr"][:].shape[1]
                K = P[f"c{l}_wr"][:].shape[2]
                R = cin * K
                sz = cin * K * K
                twr = prep.tile([cout, sz], F32, tag="twr")
                twi = prep.tile([cout, sz], F32, tag="twi")
                nc.sync.dma_start(out=twr[:], in_=ap_of(P[f"c{l}_wr"], 0, [[sz, cout], [1, sz]]))
                nc.sync.dma_start(out=twi[:], in_=ap_of(P[f"c{l}_wi"], 0, [[sz, cout], [1, sz]]))
                tn = prep.tile([cout, sz], F32, tag="tn")
                ts_ = prep.tile([cout, sz], F32, tag="ts")
                td = prep.tile([cout, sz], F32, tag="td")
                nc.vector.tensor_scalar_mul(tn[:], twi[:], -1.0)
                nc.vector.tensor_tensor(out=ts_[:], in0=twr[:], in1=twi[:], op=ALU.add)
                nc.vector.tensor_tensor(out=td[:], in0=twr[:], in1=twi[:], op=ALU.subtract)

                def stage(dst, cols, mc, col0, w):
                    # dst[kx, mc, ky*cin+c, col0+o] = w[o, c, ky, kx]; per-(ky,kx)
                    wv = w[:].rearrange("o (c ky kx) -> o c ky kx", ky=K, kx=K)
                    for ky in range(K):
                        for kx in range(K):
                            nc.sync.dma_start(
                                out=ap_of(dst, (kx * 2 + mc) * R * cols
                                          + ky * cin * cols + col0,
                                          [[1, cout], [cols, cin]]),
                                in_=wv[:, :, ky, kx])

                if l == 2:
                    stage(stm_dram[2], 96, 0, 0, twr)
                    stage(stm_dram[2], 96, 0, 32, twi)
                    stage(stm_dram[2], 96, 0, 64, ts_)
                    stage(stm_dram[2], 96, 1, 0, tn)
                    stage(stm_dram[2], 96, 1, 32, twr)
                    stage(stm_dram[2], 96, 1, 64, td)
                else:
                    stage(stm_dram[l], 128, 0, 0, twr)
                    stage(stm_dram[l], 128, 0, 64, twi)
                    stage(stm_dram[l], 128, 1, 0, tn)
                    stage(stm_dram[l], 128, 1, 64, twr)
                    stage(sts_dram[l], 64, 0, 0, ts_)
                    stage(sts_dram[l], 64, 1, 0, td)
            # L1: st1_dram[g, kxb*11+ky, col] from w[o, ky, kx=4g+kxb]
            twr = prep.tile([16, 121], F32, tag="twr1")
            twi = prep.tile([16, 121], F32, tag="twi1")
            nc.sync.dma_start(out=twr[:], in_=ap_of(P["c1_wr"], 0, [[121, 16], [1, 121]]))
            nc.sync.dma_start(out=twi[:], in_=ap_of(P["c1_wi"], 0, [[121, 16], [1, 121]]))
            ts_ = prep.tile([16, 121], F32, tag="ts1")
            nc.vector.tensor_tensor(out=ts_[:], in0=twr[:], in1=twi[:], op=ALU.add)
            for col0, w in ((0, twr), (16, twi), (32, ts_)):
                wv = w[:].rearrange("o (ky kx) -> o ky kx", kx=11)
                for kx in range(11):
                    # st1_dram row = ky*11 + kx
                    nc.sync.dma_start(
                        out=ap_of(st1_dram, kx * 64 + col0,
                                  [[1, 16], [11 * 64, 11]]),
                        in_=wv[:, :, kx])
            for kb in range(2):
                t = prep.tile([128, 1024], F32, tag="fwn")
                nc.sync.dma_start(out=t[:], in_=ap_of(P["fc1_wi"], kb * 128 * 1024,
                                                      [[1024, 128], [1, 1024]]))
                nc.vector.tensor_scalar_mul(t[:], t[:], -1.0)
                nc.sync.dma_start(out=ap_of(fc1_wineg, kb * 128 * 1024,
                                            [[1024, 128], [1, 1024]]), in_=t[:])

            # ---- conv bias vectors ----
            for l in (1, 2, 3, 4):
                C = LCFG[l]["C"]
                br = prep.tile([C, 1], F32, tag="bbr")
                bi = prep.tile([C, 1], F32, tag="bbi")
                nc.sync.dma_start(out=br[:], in_=ap_of(P[f"c{l}_br"], 0, [[1, C], [0, 1]]))
                nc.sync.dma_start(out=bi[:], in_=ap_of(P[f"c{l}_bi"], 0, [[1, C], [0, 1]]))
                b_r = prep.tile([C, 1], F32, tag="b_r")
                b_i = prep.tile([C, 1], F32, tag="b_i")
                b_s = prep.tile([C, 1], F32, tag="b_s")
                if l == 1:
                    nc.vector.tensor_tensor(out=b_r[:], in0=br[:], in1=bi[:], op=ALU.subtract)
                    nc.vector.tensor_tensor(out=b_i[:], in0=br[:], in1=bi[:], op=ALU.add)
                    nc.vector.tensor_scalar_mul(b_s[:], br[:], 2.0)
                else:
                    nc.vector.tensor_copy(b_r[:], br[:])
                    nc.vector.tensor_copy(b_i[:], bi[:])
                    nc.vector.tensor_tensor(out=b_s[:], in0=br[:], in1=bi[:], op=ALU.add)
                nc.sync.dma_start(out=ap_of(bias_dram[l], 0, [[1, C]]), in_=b_r[:, 0:1])
                nc.sync.dma_start(out=ap_of(bias_dram[l], C, [[1, C]]), in_=b_i[:, 0:1])
                nc.sync.dma_start(out=ap_of(bias_dram[l], 2 * C, [[1, C]]), in_=b_s[:, 0:1])
            # fc bias combos
            for kb in range(2):
                br = prep.tile([128, 1], F32, tag="fbr")
                bi = prep.tile([128, 1], F32, tag="fbi")
                nc.sync.dma_start(out=br[:], in_=ap_of(P["fc1_br"], kb * 128, [[1, 128], [0, 1]]))
                nc.sync.dma_start(out=bi[:], in_=ap_of(P["fc1_bi"], kb * 128, [[1, 128], [0, 1]]))
                d1 = prep.tile([128, 1], F32, tag="fd1")
                d2 = prep.tile([128, 1], F32, tag="fd2")
                nc.vector.tensor_tensor(out=d1[:], in0=br[:], in1=bi[:], op=ALU.subtract)
                nc.vector.tensor_tensor(out=d2[:], in0=br[:], in1=bi[:], op=ALU.add)
                nc.sync.dma_start(out=ap_of(fcb_dram, kb * 128, [[1, 128]]), in_=d1[:, 0:1])
                nc.sync.dma_start(out=ap_of(fcb_dram, 256 + kb * 128, [[1, 128]]), in_=d2[:, 0:1])

            def bias_tile(l, rows_spec, tag):
                """Build [total,1] per-partition bias tile.
                rows_spec: list of (comp_idx or -1, count)."""
                C = LCFG[l]["C"]
                total = sum(cnt for _, cnt in rows_spec)
                bt = singles.tile([total, 1], F32, tag=tag)
                nc.vector.memset(bt[:], 0.0)
                off = 0
                for comp, cnt in rows_spec:
                    if comp >= 0:
                        nc.gpsimd.dma_start(
                            out=bt[off:off + cnt, :],
                            in_=ap_of(bias_dram[l], comp * C, [[1, cnt], [0, 1]]))
                    off += cnt
                return bt

            # stats slot tiles
            nslotA = {1: 4 * NPER, 2: 2 * NPER, 3: NPER // 2, 4: NPER // 8}
            nslotS = {1: 2 * NPER, 2: NPER, 3: NPER // 2, 4: NPER // 8}
            sums = {l: statp.tile([128, nslotA[l]], F32, tag=f"sums{l}", name=f"sums{l}") for l in (1, 2, 3, 4)}
            sqs = {l: statp.tile([128, nslotS[l]], F32, tag=f"sqs{l}", name=f"sqs{l}") for l in (1, 2, 3, 4)}
            sqs_s = {l: statp.tile([64, nslotS[l]], F32, tag=f"sqss{l}", name=f"sqss{l}") for l in (3, 4)}

            psA = ctx.enter_context(tc.tile_pool(name="psA", bufs=3, space="PSUM"))
            psS = ctx.enter_context(tc.tile_pool(name="psS", bufs=1, space="PSUM"))

            # ================= Layer 1 phase A =================
            st1 = singles.tile([121, 64], F32, tag="st1", name="st1")
            nc.sync.dma_start(out=st1[:], in_=ap_of(st1_dram, 0, [[64, 121], [1, 64]]))
            bias1 = bias_tile(1, [(0, 16), (1, 16), (2, 16), (-1, 16),
                                  (0, 16), (1, 16), (2, 16), (-1, 16)], "bias1")

            for n in range(NPER):
              for hf in range(2):
                mv3 = mvp.tile([121, 32, 64], F32, tag="mv_0", name="mv3")
                for ky in range(11):
                    nc.gpsimd.dma_start(
                        out=mv3[11 * ky:11 * ky + 11, :, :],
                        in_=ap_of(xpad, n * 5476 + hf * 32 * 74 + ky * 74,
                                  [[1, 11], [74, 32], [1, 64]]))
                spill = spillp.tile([128, 2048], F32, tag="spill")
                for t in range(2):
                    ps = psA.tile([128, 512], F32, tag="cps")
                    for j in range(2):
                        y0 = t * 16 + j * 8  # 8 output rows per 512-px chunk
                        rhs = mv3[:, y0:y0 + 8, :]
                        nc.tensor.matmul(
                            ps[64 * j:64 * j + 64, :], st1[:], rhs,
                            start=True, stop=True,
                            tile_position=(0, 64 * j))
                    nc.vector.tensor_scalar(
                        out=spill[:, 512 * t:512 * (t + 1)], in0=ps[:],
                        scalar1=bias1[:], scalar2=0.0, op0=ALU.add, op1=ALU.add,
                        accum_out=sums[1][:, 4 * n + 2 * hf + t:4 * n + 2 * hf + t + 1])
                scr = sqp.tile([128, 2048], F32, tag="sqscr")
                nc.scalar.activation(out=scr[:, 0:1024], in_=spill[:, 0:1024],
                                     func=ACTF.Square,
                                     accum_out=sqs[1][:, 2 * n + hf:2 * n + hf + 1])
                for j in range(2):
                    nc.sync.dma_start(
                        out=ap_of(a[1], n * 4096 + hf * 2048 + j * 512,
                                  [[16 * NPER * 4096, 2], [NPER * 4096, 16],
                                   [1024, 2], [1, 512]]),
                        in_=spill[64 * j:64 * j + 32, 0:1024].rearrange(
                            "q (t c) -> q t c", t=2))

            # ================= generic stats -> AR -> coefs =================
            def finish_stats(l):
                """row_map: dict comp->(tile, row0, count[, extra (tile,row0)])
                lists of (tile, row0) partial blocks per comp (r=0,i=1,s=2),
                squares analogue. Assemble cc_in, AllReduce, compute coefs,
                broadcast into [128,6] coef tile; returns coef tile."""
                C = LCFG[l]["C"]
                # reduce slot tiles -> [rows,1]
                sredA = statp.tile([128, 1], F32, tag=f"sredA{l}")
                nc.vector.tensor_reduce(out=sredA[:], in_=sums[l][:],
                                        axis=mybir.AxisListType.X, op=ALU.add)
                sredQ = statp.tile([128, 1], F32, tag=f"sredQ{l}")
                nc.vector.tensor_reduce(out=sredQ[:], in_=sqs[l][:],
                                        axis=mybir.AxisListType.X, op=ALU.add)
                if l in (3, 4):
                    sredQs = statp.tile([64, 1], F32, tag=f"sredQs{l}")
                    nc.vector.tensor_reduce(out=sredQs[:], in_=sqs_s[l][:],
                                            axis=mybir.AxisListType.X, op=ALU.add)
                # assemble cc_in [5, C]: rows: s_r, s_i, q_rr, q_ii, q_ss
                if l == 1:
                    # psum rows (j2)(comp4:{r,i,s,junk} x16): combine j blocks on host side:
                    # write both j-halves, second with CCE-style add? -> do DVE add via aligned copy
                    tmp = statp.tile([64, 2], F32, tag="l1tmp")
                    # gather j=0 rows 0:48 -> col0 ; j=1 rows 64:112 -> col1 (DMA, then TT add)
                    nc.gpsimd.dma_start(out=tmp[0:48, 0:1], in_=sredA[0:48, :])
                    nc.gpsimd.dma_start(out=tmp[0:48, 1:2], in_=sredA[64:112, :])
                    tmq = statp.tile([64, 2], F32, tag="l1tmq")
                    nc.gpsimd.dma_start(out=tmq[0:48, 0:1], in_=sredQ[0:48, :])
                    nc.gpsimd.dma_start(out=tmq[0:48, 1:2], in_=sredQ[64:112, :])
                    totA = statp.tile([64, 1], F32, tag="l1totA")
                    totQ = statp.tile([64, 1], F32, tag="l1totQ")
                    nc.vector.tensor_tensor(out=totA[0:48, :], in0=tmp[0:48, 0:1],
                                            in1=tmp[0:48, 1:2], op=ALU.add)
                    nc.vector.tensor_tensor(out=totQ[0:48, :], in0=tmq[0:48, 0:1],
                                            in1=tmq[0:48, 1:2], op=ALU.add)
                    nc.sync.dma_start(out=ap_of(cc_in[l], 0, [[1, 2 * C]]), in_=totA[0:32, :])
                    nc.sync.dma_start(out=ap_of(cc_in[l], 2 * C, [[1, 3 * C]]), in_=totQ[0:48, :])
                elif l == 2:
                    # rows (comp3 x32): sums rows 0:64 are r,i; squares rows 0:96
                    nc.sync.dma_start(out=ap_of(cc_in[l], 0, [[1, 2 * C]]), in_=sredA[0:64, :])
                    nc.sync.dma_start(out=ap_of(cc_in[l], 2 * C, [[1, 3 * C]]), in_=sredQ[0:96, :])
                else:
                    # main rows (comp2 x64) + s tiles
                    nc.sync.dma_start(out=ap_of(cc_in[l], 0, [[1, 2 * C]]), in_=sredA[0:128, :])
                    nc.sync.dma_start(out=ap_of(cc_in[l], 2 * C, [[1, 2 * C]]), in_=sredQ[0:128, :])
                    nc.sync.dma_start(out=ap_of(cc_in[l], 4 * C, [[1, C]]), in_=sredQs[0:64, :])
                nc.gpsimd.collective_compute(
                    "AllReduce", ALU.add, replica_groups=rg,
                    ins=[cc_in[l].ap().opt()], outs=[cc_out[l].ap().opt()])
                # ---- coefficient math on [C,1] lanes ----
                st = statp.tile([C, 5], F32, tag=f"st{l}")
                nc.sync.dma_start(out=st[:], in_=ap_of(cc_out[l], 0, [[1, C], [C, 5]]))
                NTOTL = float(NTOT * LCFG[l]["HO"] * LCFG[l]["HO"])
                inv_n = 1.0 / NTOTL

                def tl(tag):
                    return statp.tile([C, 1], F32, tag=f"{tag}{l}", name=f"{tag}{l}")

                mr, mi = tl("mr"), tl("mi")
                nc.vector.tensor_scalar_mul(mr[:], st[:, 0:1], inv_n)
                nc.vector.tensor_scalar_mul(mi[:], st[:, 1:2], inv_n)
                t1, t2, t3 = tl("t1"), tl("t2"), tl("t3")
                nc.vector.tensor_scalar(out=t1[:], in0=st[:, 2:3], scalar1=inv_n,
                                        scalar2=EPS, op0=ALU.mult, op1=ALU.add)
                nc.vector.tensor_scalar(out=t2[:], in0=st[:, 3:4], scalar1=inv_n,
                                        scalar2=EPS, op0=ALU.mult, op1=ALU.add)
                # q_ss/N ; s_ri = (qss - qrr - qii)/2 computed from raw sums:
                nc.vector.tensor_scalar_mul(t3[:], st[:, 4:5], inv_n)
                w1, w2, Crr, Cii, Cri = tl("w1"), tl("w2"), tl("Crr"), tl("Cii"), tl("Cri")
                nc.vector.tensor_tensor(out=w1[:], in0=mr[:], in1=mr[:], op=ALU.mult)
                nc.vector.tensor_tensor(out=Crr[:], in0=t1[:], in1=w1[:], op=ALU.subtract)
                nc.vector.tensor_tensor(out=w1[:], in0=mi[:], in1=mi[:], op=ALU.mult)
                nc.vector.tensor_tensor(out=Cii[:], in0=t2[:], in1=w1[:], op=ALU.subtract)
                # E[ri] = (qss/N - qrr/N - qii/N)/2 ... but qrr/N etc include EPS; use raw:
                # t3 - (t1-EPS) - (t2-EPS) = t3 - t1 - t2 + 2EPS ; then /2
                nc.vector.tensor_tensor(out=w1[:], in0=t3[:], in1=t1[:], op=ALU.subtract)
                nc.vector.tensor_tensor(out=w1[:], in0=w1[:], in1=t2[:], op=ALU.subtract)
                nc.vector.tensor_scalar(out=w1[:], in0=w1[:], scalar1=0.5,
                                        scalar2=EPS, op0=ALU.mult, op1=ALU.add)
                # w1 = E[rs]... E[ri] now ; Cri = E[ri] - mr*mi
                nc.vector.tensor_tensor(out=w2[:], in0=mr[:], in1=mi[:], op=ALU.mult)
                nc.vector.tensor_tensor(out=Cri[:], in0=w1[:], in1=w2[:], op=ALU.subtract)
                det, s_, tt_, inv = tl("det"), tl("s_"), tl("tt_"), tl("inv")
                nc.vector.tensor_tensor(out=det[:], in0=Crr[:], in1=Cii[:], op=ALU.mult)
                nc.vector.tensor_tensor(out=w1[:], in0=Cri[:], in1=Cri[:], op=ALU.mult)
                nc.vector.tensor_tensor(out=det[:], in0=det[:], in1=w1[:], op=ALU.subtract)
                nc.scalar.activation(out=s_[:], in_=det[:], func=ACTF.Sqrt)
                nc.vector.tensor_tensor(out=w1[:], in0=Crr[:], in1=Cii[:], op=ALU.add)
                nc.vector.tensor_scalar(out=w2[:], in0=s_[:], scalar1=2.0, scalar2=None,
                                        op0=ALU.mult)
                nc.vector.tensor_tensor(out=w1[:], in0=w1[:], in1=w2[:], op=ALU.add)
                nc.scalar.activation(out=tt_[:], in_=w1[:], func=ACTF.Sqrt)
                nc.vector.tensor_tensor(out=w1[:], in0=s_[:], in1=tt_[:], op=ALU.mult)
                nc.vector.reciprocal(out=inv[:], in_=w1[:])
                Rrr, Rii, Rri = tl("Rrr"), tl("Rii"), tl("Rri")
                nc.vector.tensor_tensor(out=w1[:], in0=Cii[:], in1=s_[:], op=ALU.add)
                nc.vector.tensor_tensor(out=Rrr[:], in0=w1[:], in1=inv[:], op=ALU.mult)
                nc.vector.tensor_tensor(out=w1[:], in0=Crr[:], in1=s_[:], op=ALU.add)
                nc.vector.tensor_tensor(out=Rii[:], in0=w1[:], in1=inv[:], op=ALU.mult)
                nc.vector.tensor_tensor(out=w1[:], in0=Cri[:], in1=inv[:], op=ALU.mult)
                nc.vector.tensor_scalar_mul(Rri[:], w1[:], -1.0)
                # gammas / betas
                grr, gri, gii, bbr, bbi = tl("grr"), tl("gri"), tl("gii"), tl("bbr2"), tl("bbi2")
                nc.sync.dma_start(out=grr[:], in_=ap_of(P[f"b{l}_grr"], 0, [[1, C], [0, 1]]))
                nc.sync.dma_start(out=gri[:], in_=ap_of(P[f"b{l}_gri"], 0, [[1, C], [0, 1]]))
                nc.sync.dma_start(out=gii[:], in_=ap_of(P[f"b{l}_gii"], 0, [[1, C], [0, 1]]))
                nc.sync.dma_start(out=bbr[:], in_=ap_of(P[f"b{l}_br"], 0, [[1, C], [0, 1]]))
                nc.sync.dma_start(out=bbi[:], in_=ap_of(P[f"b{l}_bi"], 0, [[1, C], [0, 1]]))
                co = statp.tile([C, 6], F32, tag=f"co{l}")

                def mac2(dst, g1, R1, g2, R2):
                    nc.vector.tensor_tensor(out=w1[:], in0=g1[:], in1=R1[:], op=ALU.mult)
                    nc.vector.tensor_tensor(out=w2[:], in0=g2[:], in1=R2[:], op=ALU.mult)
                    nc.vector.tensor_tensor(out=dst, in0=w1[:], in1=w2[:], op=ALU.add)

                mac2(co[:, 0:1], grr, Rrr, gri, Rri)   # A
                mac2(co[:, 1:2], grr, Rri, gri, Rii)   # B
                mac2(co[:, 3:4], gri, Rrr, gii, Rri)   # D
                mac2(co[:, 4:5], gri, Rri, gii, Rii)   # E
                # C0 = br - A*mr - B*mi ; F0 = bi - D*mr - E*mi
                nc.vector.tensor_tensor(out=w1[:], in0=co[:, 0:1], in1=mr[:], op=ALU.mult)
                nc.vector.tensor_tensor(out=w2[:], in0=co[:, 1:2], in1=mi[:], op=ALU.mult)
                nc.vector.tensor_tensor(out=w1[:], in0=w1[:], in1=w2[:], op=ALU.add)
                nc.vector.tensor_tensor(out=co[:, 2:3], in0=bbr[:], in1=w1[:], op=ALU.subtract)
                nc.vector.tensor_tensor(out=w1[:], in0=co[:, 3:4], in1=mr[:], op=ALU.mult)
                nc.vector.tensor_tensor(out=w2[:], in0=co[:, 4:5], in1=mi[:], op=ALU.mult)
                nc.vector.tensor_tensor(out=w1[:], in0=w1[:], in1=w2[:], op=ALU.add)
                nc.vector.tensor_tensor(out=co[:, 5:6], in0=bbi[:], in1=w1[:], op=ALU.subtract)
                nc.sync.dma_start(out=ap_of(coef_dram[l], 0, [[1, C * 6]]), in_=co[:])
                G = 128 // (2 * C) * 2  # images per phase-D lane group
                coefb = singles.tile([128, 6], F32, tag=f"coefb{l}")
                nc.gpsimd.dma_start(
                    out=coefb[0:G * C, :],
                    in_=ap_of(coef_dram[l], 0, [[0, G], [6, C], [1, 6]]))
                return coefb

            coefb1 = finish_stats(1)

            # ================= phase D (normalize+relu+pool) =================
            def phase_d(l, coefb):
                C = LCFG[l]["C"]
                HO = LCFG[l]["HO"]
                HQ = HO // 2
                G = 128 // C          # images per chunk held in lanes
                NF = {1: 1, 2: 1, 3: 4, 4: 8}[l]   # images per lane via free dim
                YW = {1: 16, 2: 32, 3: 16, 4: 8}[l]  # y-window rows
                CN = a[l][:].shape[3]  # px per image
                WIN = YW * HO
                nimg_chunk = G * NF
                nych = HO // YW
                A_ = coefb[:, 0:1]
                B_ = coefb[:, 1:2]
                C0 = coefb[:, 2:3]
                D_ = coefb[:, 3:4]
                E_ = coefb[:, 4:5]
                F0 = coefb[:, 5:6]
                for n0 in range(0, NPER, nimg_chunk):
                    for yw in range(nych):
                        tin = dp.tile([128, 2, NF, WIN], F32, tag="din")
                        for g in range(G):
                            nc.gpsimd.dma_start(
                                out=tin[g * C:(g + 1) * C, :, :, :],
                                in_=ap_of(a[l], (n0 + g * NF) * CN + yw * WIN,
                                          [[NPER * CN, C], [C * NPER * CN, 2],
                                           [CN, NF], [1, WIN]]))
                        r = tin[:, 0]
                        i_ = tin[:, 1]
                        yr = dtmp.tile([128, NF, WIN], F32, tag="yr")
                        yi = dtmp.tile([128, NF, WIN], F32, tag="yi")
                        tmp = dtmp.tile([128, NF, WIN], F32, tag="tmp")
                        nc.vector.tensor_scalar(out=yr[:], in0=r, scalar1=A_,
                                                scalar2=C0, op0=ALU.mult, op1=ALU.add)
                        nc.vector.tensor_scalar(out=tmp[:], in0=i_, scalar1=B_,
                                                scalar2=None, op0=ALU.mult)
                        nc.vector.tensor_tensor(out=yr[:], in0=yr[:], in1=tmp[:], op=ALU.add)
                        nc.vector.tensor_scalar(out=yi[:], in0=i_, scalar1=E_,
                                                scalar2=F0, op0=ALU.mult, op1=ALU.add)
                        nc.vector.tensor_scalar(out=tmp[:], in0=r, scalar1=D_,
                                                scalar2=None, op0=ALU.mult)
                        nc.vector.tensor_tensor(out=yi[:], in0=yi[:], in1=tmp[:], op=ALU.add)
                        nc.scalar.activation(out=yr[:], in_=yr[:], func=ACTF.Relu)
                        nc.scalar.activation(out=yi[:], in_=yi[:], func=ACTF.Relu)
                        # mag^2
                        m = dtmp.tile([128, NF, WIN], F32, tag="m")
                        nc.scalar.activation(out=m[:], in_=yr[:], func=ACTF.Square)
                        nc.scalar.activation(out=tmp[:], in_=yi[:], func=ACTF.Square)
                        nc.vector.tensor_tensor(out=m[:], in0=m[:], in1=tmp[:], op=ALU.add)
                        # 2x2 magnitude-argmax pool
                        NYW = YW // 2
                        HQW = HO // 2
                        FY = NF * NYW

                        def quad(t):
                            return t[:].rearrange(
                                "p f (y two x tw) -> p (f y) two x tw",
                                two=2, tw=2, y=NYW)

                        mq, rq, iq = quad(m), quad(yr), quad(yi)
                        mh = dtmp.tile([128, FY, 2, HQW], F32, tag="mh")
                        msk = dtmp.tile([128, FY, 2, HQW], mybir.dt.uint8, tag="msk")
                        rh = dtmp.tile([128, FY, 2, HQW], F32, tag="rh")
                        ih = dtmp.tile([128, FY, 2, HQW], F32, tag="ih")
                        # horizontal: compare x-even vs x-odd (both rows of pair)
                        nc.vector.tensor_tensor(out=msk[:], in0=mq[:, :, :, :, 0],
                                                in1=mq[:, :, :, :, 1], op=ALU.is_ge)
                        nc.vector.tensor_tensor(out=mh[:], in0=mq[:, :, :, :, 0],
                                                in1=mq[:, :, :, :, 1], op=ALU.max)
                        nc.vector.tensor_copy(rh[:], rq[:, :, :, :, 1])
                        nc.vector.copy_predicated(rh[:], msk[:], rq[:, :, :, :, 0])
                        nc.vector.tensor_copy(ih[:], iq[:, :, :, :, 1])
                        nc.vector.copy_predicated(ih[:], msk[:], iq[:, :, :, :, 0])
                        # vertical: row 0 vs row 1 of each pair
                        msv = dtmp.tile([128, FY, HQW], mybir.dt.uint8, tag="msv")
                        nc.vector.tensor_tensor(out=msv[:], in0=mh[:, :, 0, :],
                                                in1=mh[:, :, 1, :], op=ALU.is_ge)
                        pr = dout.tile([128, NF, NYW, HQW], F32, tag="pr")
                        pi = dout.tile([128, NF, NYW, HQW], F32, tag="pi")
                        prv = pr[:].rearrange("p f y x -> p (f y) x")
                        piv = pi[:].rearrange("p f y x -> p (f y) x")
                        nc.vector.tensor_copy(prv, rh[:, :, 1, :])
                        nc.vector.copy_predicated(prv, msv[:], rh[:, :, 0, :])
                        nc.vector.tensor_copy(piv, ih[:, :, 1, :])
                        nc.vector.copy_predicated(piv, msv[:], ih[:, :, 0, :])
                        # write to p[l] padded interior (or p4 flat)
                        y0 = yw * NYW
                        if l < 4:
                            PD = {1: 2, 2: 1, 3: 1}[l]
                            HP1 = {1: 36, 2: 18, 3: 10}[l]
                            NPX1 = HP1 * HP1
                            for comp, src in ((0, pr), (1, pi)):
                                for g in range(G):
                                    for nf in range(NF):
                                        nc.sync.dma_start(
                                            out=ap_of(p[l], comp * C * NPER * NPX1
                                                      + (n0 + g * NF + nf) * NPX1
                                                      + (PD + y0) * HP1 + PD,
                                                      [[NPER * NPX1, C],
                                                       [HP1, NYW], [1, HQW]]),
                                            in_=src[g * C:(g + 1) * C, nf, :, :])
                        else:
                            # p4 [2, n, feat]: per (g, nf) one contiguous 1024-vec
                            for comp, src in ((0, pr), (1, pi)):
                                for g in range(G):
                                    for nf in range(NF):
                                        nc.sync.dma_start(
                                            out=ap_of(p[4], comp * 1024 * NPER
                                                      + (n0 + g * NF + nf) * 1024,
                                                      [[16, C], [4, NYW], [1, HQW]]),
                                            in_=src[g * C:(g + 1) * C, nf, :, :])

            phase_d(1, coefb1)

            # ================= Layers 2..4 phase A =================
            def conv_a(l):
                C = LCFG[l]["C"]
                cin = {2: 16, 3: 32, 4: 64}[l]
                K = {2: 5, 3: 3, 4: 3}[l]
                HO = LCFG[l]["HO"]
                HP = {2: 36, 3: 18, 4: 10}[l]
                src = p[l - 1]
                NPX = HP * HP
                cstride = NPER * NPX
                compstride = cin * cstride

                if l == 2:
                    # staged stationary [80, 96] per (kx, movcomp); rows ky-outer
                    sts = {}
                    for kx in range(5):
                        for mc in range(2):
                            st = singles.tile([80, 96], F32, tag=f"st2_{kx}_{mc}")
                            nc.gpsimd.dma_start(
                                out=st[:],
                                in_=ap_of(stm_dram[2], (kx * 2 + mc) * 80 * 96,
                                          [[96, 80], [1, 96]]))
                            sts[(kx, mc)] = st
                    bias2 = bias_tile(2, [(0, 32), (1, 32), (2, 32)], "bias2")
                    for n in range(NPER):
                        mvs = []
                        for comp in range(2):
                            mv = mvp.tile([80, 32, 36], F32, tag=f"mv_{comp}")
                            for ky in range(5):
                                nc.gpsimd.dma_start(
                                    out=mv[16 * ky:16 * ky + 16, :, :],
                                    in_=ap_of(src, comp * compstride + n * NPX + ky * 36,
                                              [[cstride, 16], [36, 32], [1, 36]]))
                            mvs.append(mv)
                        spill2t = spillp.tile([96, 2048], F32, tag="spill", name="spill2t")
                        spill = spill2t[:, 0:1024]
                        for h in range(2):  # y halves
                            ps = psA.tile([96, 512], F32, tag="cps")
                            first = True
                            for kx in range(5):
                                for mc in range(2):
                                    rhs = mvs[mc][:, h * 16:h * 16 + 16, kx:kx + 32]
                                    nc.tensor.matmul(
                                        ps[:], sts[(kx, mc)][:], rhs,
                                        start=first, stop=(kx == 4 and mc == 1))
                                    first = False
                            nc.vector.tensor_scalar(
                                out=spill[:, 512 * h:512 * (h + 1)], in0=ps[:],
                                scalar1=bias2[:], scalar2=0.0, op0=ALU.add, op1=ALU.add,
                                accum_out=sums[2][0:96, 2 * n + h:2 * n + h + 1])
                        scr2t = sqp.tile([96, 2048], F32, tag="sqscr", name="scr2t")
                        scr = scr2t[:, 0:1024]
                        nc.scalar.activation(out=scr, in_=spill, func=ACTF.Square,
                                             accum_out=sqs[2][0:96, n:n + 1])
                        nc.sync.dma_start(
                            out=ap_of(a[2], n * 1024,
                                      [[32 * NPER * 1024, 2], [NPER * 1024, 32], [1, 1024]]),
                            in_=spill[0:64, :])
                    return

                # l in (3,4): main stationary [rows,128]=(r|i), s stationary [rows,64]
                kys = [(0, K)] if cin * K <= 128 else [(0, 2), (2, 1)]
                sts_main = {}
                sts_s = {}
                R = cin * K
                for kx in range(K):
                    for mc in range(2):
                        for gi, (ky0, kyn) in enumerate(kys):
                            rows = cin * kyn
                            st = singles.tile([rows, 128], F32, tag=f"st{l}m_{kx}_{mc}_{gi}")
                            nc.gpsimd.dma_start(
                                out=st[:],
                                in_=ap_of(stm_dram[l],
                                          (kx * 2 + mc) * R * 128 + ky0 * cin * 128,
                                          [[128, rows], [1, 128]]))
                            sts_main[(kx, mc, gi)] = st
                            st2 = singles.tile([rows, 64], F32, tag=f"st{l}s_{kx}_{mc}_{gi}")
                            nc.gpsimd.dma_start(
                                out=st2[:],
                                in_=ap_of(sts_dram[l],
                                          (kx * 2 + mc) * R * 64 + ky0 * cin * 64,
                                          [[64, rows], [1, 64]]))
                            sts_s[(kx, mc, gi)] = st2
                biasM = bias_tile(l, [(0, 64), (1, 64)], f"biasM{l}")
                biasS = bias_tile(l, [(2, 64)], f"biasS{l}")
                NI = 2 if l == 3 else 8   # images per psum tile
                PXI = HO * HO
                for n0 in range(0, NPER, NI):
                    mvs = {}
                    for comp in range(2):
                        for gi, (ky0, kyn) in enumerate(kys):
                            rows = cin * kyn
                            mv = mvp.tile([rows, NI, HO, HP], F32,
                                          tag=(f"mv_{comp}" if gi == 0 else f"mvb_{comp}"))
                            for ki in range(kyn):
                                nc.gpsimd.dma_start(
                                    out=mv[ki * cin:(ki + 1) * cin, :, :, :],
                                    in_=ap_of(src, comp * compstride + n0 * NPX
                                              + (ky0 + ki) * HP,
                                              [[cstride, cin],
                                               [NPX, NI], [HP, HO], [1, HP]]))
                            mvs[(comp, gi)] = mv
                    psm = psA.tile([128, 512], F32, tag="cps")
                    pss = psS.tile([64, 512], F32, tag="sps")
                    nmm = len(kys) * 2 * K
                    for target, stdict, pstile in ((0, sts_main, psm), (1, sts_s, pss)):
                        if l == 3:
                            for ii in range(NI):
                                cnt = 0
                                for kx in range(K):
                                    for mc in range(2):
                                        for gi in range(len(kys)):
                                            mv = mvs[(mc, gi)]
                                            rhs = mv[:, ii, :, kx:kx + HO]
                                            cnt += 1
                                            nc.tensor.matmul(
                                                pstile[:, ii * PXI:(ii + 1) * PXI],
                                                stdict[(kx, mc, gi)][:], rhs,
                                                start=(cnt == 1), stop=(cnt == nmm))
                        else:
                            cnt = 0
                            for kx in range(K):
                                for mc in range(2):
                                    for gi in range(len(kys)):
                                        mv = mvs[(mc, gi)]
                                        rhs = mv[:, :, :, kx:kx + HO]
                                        cnt += 1
                                        nc.tensor.matmul(
                                            pstile[:], stdict[(kx, mc, gi)][:], rhs,
                                            start=(cnt == 1), stop=(cnt == nmm))
                    g = n0 // NI
                    spill34t = spillp.tile([128, 2048], F32, tag="spill", name="spill34t")
                    spill = spill34t[:, 0:512]
                    nc.vector.tensor_scalar(
                        out=spill, in0=psm[:], scalar1=biasM[:], scalar2=0.0,
                        op0=ALU.add, op1=ALU.add, accum_out=sums[l][:, g:g + 1])
                    scr34t = sqp.tile([128, 2048], F32, tag="sqscr", name="scr34t")
                    scr = scr34t[:, 0:512]
                    nc.scalar.activation(out=scr, in_=spill, func=ACTF.Square,
                                         accum_out=sqs[l][:, g:g + 1])
                    scr2 = sqp.tile([64, 512], F32, tag="sqscrs")
                    nc.scalar.activation(out=scr2[:], in_=pss[:], func=ACTF.Square,
                                         bias=biasS[:], scale=1.0,
                                         accum_out=sqs_s[l][:, g:g + 1])
                    for ii in range(NI):
                        nc.sync.dma_start(
                            out=ap_of(a[l], (n0 + ii) * PXI,
                                      [[64 * NPER * PXI, 2], [NPER * PXI, 64], [1, PXI]]),
                            in_=spill[:, ii * PXI:(ii + 1) * PXI])

            conv_a(2)
            coefb2 = finish_stats(2)
            phase_d(2, coefb2)
            conv_a(3)
            coefb3 = finish_stats(3)
            phase_d(3, coefb3)
            conv_a(4)
            coefb4 = finish_stats(4)
            phase_d(4, coefb4)

            # ================= FC head (fp32) =================
            fcp = ctx.enter_context(tc.tile_pool(name="fcp", bufs=2))
            psF = ctx.enter_context(tc.tile_pool(name="psF", bufs=2, space="PSUM"))
            psT = ctx.enter_context(tc.tile_pool(name="psT", bufs=2, space="PSUM"))
            ident = singles.tile([128, 128], F32, tag="ident")
            make_identity(nc, ident[:])
            # moving: fr/fi chunks [128,64]
            mvF = {}
            for comp in range(2):
                for kb in range(8):
                    t = fcp.tile([128, NPER], F32, tag=f"mvF{comp}_{kb}")
                    nc.sync.dma_start(
                        out=t[:],
                        in_=ap_of(p[4], comp * 1024 * NPER + kb * 128,
                                  [[1, 128], [1024, NPER]]))
                    mvF[(comp, kb)] = t
            fcbias = {}
            for comp in range(2):
                for ob in range(2):
                    bt = singles.tile([128, 1], F32, tag=f"fcb{comp}_{ob}")
                    nc.sync.dma_start(out=bt[:], in_=ap_of(fcb_dram, comp * 256 + ob * 128,
                                                           [[1, 128], [0, 1]]))
                    fcbias[(comp, ob)] = bt
            embs = {}
            for comp in range(2):  # 0: er, 1: ei
                for ob in range(2):
                    ps = psF.tile([128, NPER], F32, tag="fps")
                    cnt = 0
                    for kb in range(8):
                        for mc in range(2):
                            # er: fr*Wr + fi*(-Wi) ; ei: fi*Wr + fr*Wi
                            if comp == 0:
                                wsrc = P["fc1_wr"] if mc == 0 else fc1_wineg
                                mov = mvF[(mc, kb)]
                            else:
                                wsrc = P["fc1_wr"] if mc == 0 else P["fc1_wi"]
                                mov = mvF[(1 - mc, kb)]
                            st = fcp.tile([128, 128], F32, tag=f"stF")
                            nc.sync.dma_start(
                                out=st[:],
                                in_=ap_of(wsrc, ob * 128 * 1024 + kb * 128,
                                          [[1, 128], [1024, 128]]))
                            cnt += 1
                            nc.tensor.matmul(ps[:], st[:], mov[:],
                                             start=(cnt == 1), stop=(cnt == 16))
                    emb = fcp.tile([128, NPER], F32, tag=f"emb{comp}_{ob}")
                    # relu(x + bias) during evac
                    nc.scalar.activation(out=emb[:], in_=ps[:], func=ACTF.Relu,
                                         bias=fcbias[(comp, ob)][:], scale=1.0)
                    embs[(comp, ob)] = emb
                    # transpose -> [64, 128] -> DMA out
                    pst = psT.tile([128, 128], F32, tag="pst")
                    nc.tensor.transpose(pst[0:NPER, :], emb[:], ident[:])
                    ev = fcp.tile([NPER, 128], F32, tag="evT")
                    nc.vector.tensor_copy(ev[:], pst[0:NPER, 0:128])
                    dst = out_er if comp == 0 else out_ei
                    nc.sync.dma_start(
                        out=ap_of(dst, ob * 128, [[256, NPER], [1, 128]]),
                        in_=ev[:])
            # mag = sqrt(er^2 + ei^2 + eps)
            epst = singles.tile([128, 1], F32, tag="epst")
            nc.vector.memset(epst[:], MAG_EPS)
            mags = {}
            for ob in range(2):
                sq1 = fcp.tile([128, NPER], F32, tag="msq1")
                sq2 = fcp.tile([128, NPER], F32, tag="msq2")
                nc.scalar.activation(out=sq1[:], in_=embs[(0, ob)][:], func=ACTF.Square)
                nc.scalar.activation(out=sq2[:], in_=embs[(1, ob)][:], func=ACTF.Square)
                nc.vector.tensor_tensor(out=sq1[:], in0=sq1[:], in1=sq2[:], op=ALU.add)
                mg = fcp.tile([128, NPER], F32, tag=f"mag{ob}")
                nc.scalar.activation(out=mg[:], in_=sq1[:], func=ACTF.Sqrt,
                                     bias=epst[:], scale=1.0)
                mags[ob] = mg
            # fc2
            for ob in range(8):
                ps = psF.tile([125, NPER], F32, tag="fps")
                for kb in range(2):
                    st = fcp.tile([128, 125], F32, tag="stF2")
                    nc.sync.dma_start(
                        out=st[:],
                        in_=ap_of(P["fc2_w"], ob * 125 * 256 + kb * 128,
                                  [[1, 128], [256, 125]]))
                    nc.tensor.matmul(ps[:], st[:], mags[kb][:],
                                     start=(kb == 0), stop=(kb == 1))
                b2 = fcp.tile([125, 1], F32, tag="b2t")
                nc.sync.dma_start(out=b2[:], in_=ap_of(P["fc2_b"], ob * 125,
                                                       [[1, 125], [0, 1]]))
                lg = fcp.tile([125, NPER], F32, tag="lg")
                nc.vector.tensor_scalar(out=lg[:], in0=ps[:], scalar1=b2[:],
                                        scalar2=None, op0=ALU.add)
                pst = psT.tile([128, 128], F32, tag="pst")
                nc.tensor.transpose(pst[0:NPER, 0:125], lg[:], ident[0:125, 0:125])
                ev = fcp.tile([NPER, 125], F32, tag="evT2")
                nc.vector.tensor_copy(ev[:], pst[0:NPER, 0:125])
                nc.sync.dma_start(
                    out=ap_of(out_logits, ob * 125, [[1000, NPER], [1, 125]]),
                    in_=ev[:])

            if DEBUG:
                def dcopy(dst_t, src_t, total):
                    nc.sync.dma_start(out=ap_of(dst_t, 0, [[1, total]]),
                                      in_=ap_of(src_t, 0, [[1, total]]))
                dcopy(dbg["a1"], a[1], 2 * 16 * NPER * 4096)
                dcopy(dbg["p1"], p[1], 2 * 16 * NPER * 1296)
                dcopy(dbg["a2"], a[2], 2 * 32 * NPER * 1024)
                dcopy(dbg["p4"], p[4], 2 * NPER * 1024)
                for l in (1, 2, 3, 4):
                    C = LCFG[l]["C"]
                    dcopy(dbg[f"cc{l}"], cc_out[l], 5 * C)
                    dcopy(dbg[f"co{l}"], coef_dram[l], C * 6)

    nc.compile()
    return nc


_NC_CACHE = None


def kernel(**inputs):
    global _NC_CACHE
    if _NC_CACHE is None:
        _NC_CACHE = build()
    nc = _NC_CACHE
    xs = np.ascontiguousarray(inputs["x"].astype(np.float32))
    in_maps = []
    for c in range(NCORES):
        m = {k: np.ascontiguousarray(np.asarray(v, dtype=np.float32))
             for k, v in inputs.items() if k != "x"}
        m["x"] = xs[c * NPER:(c + 1) * NPER]
        in_maps.append(m)
    res = run_bass_kernel_spmd(nc, in_maps, core_ids=list(range(NCORES)))
    logits = np.concatenate([res.results[c]["logits"] for c in range(NCORES)], 0)
    er = np.concatenate([res.results[c]["er"] for c in range(NCORES)], 0)
    ei = np.concatenate([res.results[c]["ei"] for c in range(NCORES)], 0)
    if os.environ.get("KERNEL_DEBUG", "0") != "0":
        kernel._dbg = res.results
    return logits, er, ei
